# revision 5
# baseline (speedup 1.0000x reference)
"""Trainium2 Bass kernel for nn_Attention_4045859193206 (Swin-style window
attention with relative position bias + key masking).

Contract: kernel(**inputs) takes FULL inputs (B=128 windows), shards the
batch across 8 NeuronCores (16 windows each), runs one SPMD Bass kernel,
returns the FULL (128, 196, 512) float32 output. Self-contained.

Design (variant "pair2"; TimelineSim 318us/core vs 492us for the staged v1):
  - RPE bias gathered on HOST (numpy) and shipped as a [128, 2, 196, 16]
    fp16 table in S^T layout; kills the on-device gpsimd dma_gather
    (~71us DMA + gather) and the idx stream.
  - Key mask applied by zeroing V rows and using mask-columns instead of
    ones for the Z (denominator) matmul -- exactly equivalent to -inf
    masking, and makes exp() window-independent (constant -4.0 bias only).
  - S^T computed per HEAD PAIR into 2-bank PSUM tiles, double-buffered
    (ps_s bufs=2), so each pair's ScalarE exp [128, 2, 392] overlaps the
    next pair's bias+QK matmuls. o/z PSUM pool split from the mm/y pool so
    attention tiles don't contend with the next group's QKV tiles.
  - k-chunks are (128, 68) instead of (98, 98): QK/V c0 chunks get
    M=128 stationary tiles (fast-weight-load eligible).
  - bias injected by full-array ident matmuls (start=True opens each bank
    ONCE -- first_mm clears the whole bank; QK then accumulates with
    start=False). The full-array ident also serializes bias before the
    row-packed QK on HW; packed-bias alternatives race (overlapping
    outputs from concurrent tiles) and produce NaN.
  - Q^T/K^T built per group-half in separate tiles so windows 0-1 of each
    4-window group unblock after half the QKV matmuls; QKV bias adds on
    VectorE (nc.any put them on the exp-critical ScalarE).
  - fp16 everywhere: fp8 measured infeasible (0.02..0.07 rel err vs the
    2e-2 gate on every stage combination).
"""

import contextlib
import numpy as np

import concourse.bass as bass
import concourse.mybir as mybir
import concourse.tile as tile
from concourse.bacc import Bacc

# ---------------------------------------------------------------- constants
B, N, DIM, H = 128, 196, 512, 16
HD = DIM // H                     # 32
RPE = 729
NCORES = 8
W = B // NCORES                   # 16 windows per core
GW = 4                            # windows per qkv group (4*196=784 tokens)
KC = (128, 68)                    # k chunks per window
F16 = mybir.dt.float16
F32 = mybir.dt.float32
EXP_SHIFT = -4.0                  # exp(s-4): fp16 headroom; cancels in softmax


def _build_nc(n_w=W, variant="pair2", bufs=None):
    _b = dict(xt=2, qk=4, v=2, p=3, o=2, y=3, rz=4)
    _b.update(bufs or {})
    bufs = _b
    assert n_w % GW == 0
    ngrp = n_w // GW
    nc = Bacc("TRN2", target_bir_lowering=False)

    x_d = nc.dram_tensor("x", (n_w, N, DIM), F32, kind="ExternalInput")
    wqk_d = nc.dram_tensor("wqk", (128, 4 * 2 * DIM), F16, kind="ExternalInput")
    wv_d = nc.dram_tensor("wv", (128, 4 * DIM), F16, kind="ExternalInput")
    wp_d = nc.dram_tensor("wp", (128, 4 * DIM), F16, kind="ExternalInput")
    bqk_d = nc.dram_tensor("bqk", (128, 8), F32, kind="ExternalInput")
    bv_d = nc.dram_tensor("bv", (DIM,), F32, kind="ExternalInput")
    bp_d = nc.dram_tensor("bp", (DIM,), F32, kind="ExternalInput")
    biasT_d = nc.dram_tensor("biasT", (128, 2 * N * H), F16,
                             kind="ExternalInput")
    mones_d = nc.dram_tensor("mones", (128, n_w * 2 * HD), F16,
                             kind="ExternalInput")
    maskv_d = nc.dram_tensor("maskv", (128, n_w * 2), F32,
                             kind="ExternalInput")
    ident_d = nc.dram_tensor("ident", (128, 128), F16, kind="ExternalInput")
    out_d = nc.dram_tensor("out", (n_w, N, DIM), F32, kind="ExternalOutput")

    x16_d = nc.dram_tensor("x16", (n_w * N, DIM), F16)

    with tile.TileContext(nc) as tc, contextlib.ExitStack() as ctx:
        const = ctx.enter_context(tc.tile_pool(name="const", bufs=1))
        xt_pool = ctx.enter_context(tc.tile_pool(name="xt", bufs=bufs["xt"]))
        qk_pool = ctx.enter_context(tc.tile_pool(name="qk", bufs=bufs["qk"]))
        v_pool = ctx.enter_context(tc.tile_pool(name="v", bufs=bufs["v"]))
        p_pool = ctx.enter_context(tc.tile_pool(name="p", bufs=bufs["p"]))
        o_pool = ctx.enter_context(tc.tile_pool(name="o", bufs=bufs["o"]))
        y_pool = ctx.enter_context(tc.tile_pool(name="y", bufs=bufs["y"]))
        rz_pool = ctx.enter_context(tc.tile_pool(name="rz", bufs=bufs["rz"]))
        ps_s = ctx.enter_context(tc.tile_pool(
            name="ps_s", bufs=(2 if variant in ("pair", "pair2", "pair3") else 1), space="PSUM"))
        ps_a = ctx.enter_context(tc.tile_pool(
            name="ps_a", bufs=(2 if variant in ("pair2", "pair3") else 4), space="PSUM"))
        ps_oz = ctx.enter_context(tc.tile_pool(
            name="ps_oz", bufs=2, space="PSUM")) if variant in ("pair2", "pair3") else ps_a

        # ---------------- constants ----------------
        wqk_sb = const.tile([128, 4, 2 * DIM], F16)   # [c128, ci, o] (q|k)
        wv_sb = const.tile([128, 4, DIM], F16)
        wp_sb = const.tile([128, 4, DIM], F16)
        nc.sync.dma_start(out=wqk_sb, in_=wqk_d[:].rearrange("b (a c) -> b a c", a=4))
        nc.sync.dma_start(out=wv_sb, in_=wv_d[:].rearrange("b (a c) -> b a c", a=4))
        nc.sync.dma_start(out=wp_sb, in_=wp_d[:].rearrange("b (a c) -> b a c", a=4))
        ident_sb = const.tile([128, 128], F16)
        nc.sync.dma_start(out=ident_sb, in_=ident_d[:])
        bqk_sb = const.tile([128, 8], F32)            # per-partition qk bias
        nc.sync.dma_start(out=bqk_sb, in_=bqk_d[:])
        bv_bc = const.tile([128, DIM], F32)           # broadcast rows
        nc.sync.dma_start(
            out=bv_bc, in_=bass.AP(tensor=bv_d[:].tensor, offset=0,
                                   ap=[[0, 128], [1, DIM]]))
        bp_bc = const.tile([128, DIM], F32)
        nc.sync.dma_start(
            out=bp_bc, in_=bass.AP(tensor=bp_d[:].tensor, offset=0,
                                   ap=[[0, 128], [1, DIM]]))
        # biasT_sb[p, c', q, h] = rpe bias for k = 128c'+p
        biasT_sb = const.tile([128, 2, N, H], F16)
        nc.sync.dma_start(
            out=biasT_sb,
            in_=biasT_d[:].rearrange("p (c q h) -> p c q h", c=2, q=N))
        mones_sb = const.tile([128, n_w, 2, HD], F16)
        nc.sync.dma_start(
            out=mones_sb,
            in_=mones_d[:].rearrange("p (w c d) -> p w c d", w=n_w, c=2))
        maskv_sb = const.tile([128, n_w, 2], F32)
        nc.sync.dma_start(
            out=maskv_sb,
            in_=maskv_d[:].rearrange("p (w c) -> p w c", w=n_w))
        shift_sb = const.tile([128, 1], F32)
        nc.vector.memset(shift_sb, EXP_SHIFT)

        # ---------------- main loop over 4-window groups ----------------
        for g in range(ngrp):
            tok0 = g * GW * N
            nc.gpsimd.dma_start(
                out=x16_d[tok0:tok0 + GW * N, :],
                in_=x_d[:].rearrange("w n c -> (w n) c")[tok0:tok0 + GW * N, :],
            )
            xt = xt_pool.tile([128, 4, GW * N], F16, tag="xt")
            for ci in range(4):
                nc.sync.dma_start_transpose(
                    out=xt[:, ci, :],
                    in_=x16_d[tok0:tok0 + GW * N, ci * 128:(ci + 1) * 128],
                )

            # Q^T / K^T  [o-chunk 128, tok] fp16 (q pre-scaled via weights);
            # one tile per group-half so windows 0-1 unblock after half the
            # QKV matmuls
            qkh = [qk_pool.tile([128, 8, 2 * N], F16, tag=f"qk{h}",
                                name=f"qkh{h}") for h in range(2)]
            for half in range(2):
                for oc in range(8):
                    mm_ps = ps_a.tile([128, 512], F32, tag="ps_a")
                    for ci in range(4):
                        nc.tensor.matmul(
                            mm_ps[:, 0:392],
                            lhsT=wqk_sb[:, ci, oc * 128:(oc + 1) * 128],
                            rhs=xt[:, ci, half * 392:(half + 1) * 392],
                            start=(ci == 0), stop=(ci == 3),
                        )
                    nc.vector.tensor_scalar_add(
                        out=qkh[half][:, oc, :],
                        in0=mm_ps[:, 0:392],
                        scalar1=bqk_sb[:, oc:oc + 1],
                    )

            # V natural [k-chunk, 512] fp16, bias-added then mask-zeroed
            v_sb = v_pool.tile([128, GW, 2, DIM], F16, tag="v")
            for wi in range(GW):
                w_abs = g * GW + wi
                for cp in range(2):
                    kc = KC[cp]
                    vv_ps = ps_a.tile([128, 512], F32, tag="ps_a")
                    for ci in range(4):
                        nc.tensor.matmul(
                            vv_ps[0:kc, :],
                            lhsT=xt[:, ci, wi * N + cp * 128:
                                    wi * N + cp * 128 + kc],
                            rhs=wv_sb[:, ci, :],
                            start=(ci == 0), stop=(ci == 3),
                        )
                    nc.vector.tensor_add(
                        out=v_sb[0:kc, wi, cp, :],
                        in0=vv_ps[0:kc, :],
                        in1=bv_bc[0:kc, :],
                    )
                    nc.vector.tensor_scalar_mul(
                        out=v_sb[0:kc, wi, cp, :],
                        in0=v_sb[0:kc, wi, cp, :],
                        scalar1=maskv_sb[0:kc, w_abs, cp:cp + 1],
                    )

            # ---------------- attention per window ----------------
            for wi in range(GW):
                w_abs = g * GW + wi
                oT = o_pool.tile([128, 4, N], F16, tag="oT")
                qk_sb = qkh[wi // 2]
                w0 = (wi % 2) * N
                if variant in ("pair", "pair2", "pair3"):
                    # head pairs: 2-bank S tiles, double-buffered so the
                    # next pair's PE matmuls overlap this pair's exp()
                    for hg in range(4):
                        o_ps = ps_oz.tile([128, 512], F32, tag="ps_oz")
                        z_ps = ps_oz.tile([128, 512], F32, tag="ps_oz")
                        for pr in range(2):
                            s_ps = ps_s.tile([128, 2, 512], F32, tag="s")
                            for cp in range(2):
                                kc = KC[cp]
                                for j in range(2):
                                    i = 2 * pr + j
                                    nc.tensor.matmul(
                                        s_ps[0:kc, j, cp * N:(cp + 1) * N],
                                        lhsT=ident_sb[0:kc, 0:kc],
                                        rhs=biasT_sb[0:kc, cp, :, 4 * hg + i],
                                        start=(cp == 0), stop=False,
                                        skip_group_check=True,
                                    )
                            for cp in range(2):
                                kc = KC[cp]
                                for j in range(2):
                                    i = 2 * pr + j
                                    nc.tensor.matmul(
                                        s_ps[0:kc, j, cp * N:(cp + 1) * N],
                                        lhsT=qk_sb[32 * i:32 * (i + 1),
                                                   4 + hg,
                                                   w0 + cp * 128:
                                                   w0 + cp * 128 + kc],
                                        rhs=qk_sb[32 * i:32 * (i + 1), hg,
                                                  w0:w0 + N],
                                        start=False, stop=(cp == 1),
                                        tile_position=(32 * i, 0),
                                        skip_group_check=True,
                                    )
                            p_sb = p_pool.tile([128, 2, 2 * N], F16, tag="p")
                            if variant == "pair3":
                                for cp in range(2):
                                    kc = KC[cp]
                                    nc.scalar.activation(
                                        out=p_sb[0:kc, :,
                                                 cp * N:(cp + 1) * N],
                                        in_=s_ps[0:kc, :,
                                                 cp * N:(cp + 1) * N],
                                        func=mybir.ActivationFunctionType.Exp,
                                        bias=shift_sb[0:kc, 0:1],
                                        scale=1.0,
                                    )
                            else:
                                nc.scalar.activation(
                                    out=p_sb[:],
                                    in_=s_ps[:, :, 0:2 * N],
                                    func=mybir.ActivationFunctionType.Exp,
                                    bias=shift_sb[:, 0:1],
                                    scale=1.0,
                                )
                            for j in range(2):
                                i = 2 * pr + j
                                h = 4 * hg + i
                                for cp in range(2):
                                    kc = KC[cp]
                                    nc.tensor.matmul(
                                        o_ps[32 * i:32 * (i + 1), 0:N],
                                        lhsT=v_sb[0:kc, wi, cp,
                                                  32 * h:32 * (h + 1)],
                                        rhs=p_sb[0:kc, j,
                                                 cp * N:(cp + 1) * N],
                                        start=(cp == 0), stop=(cp == 1),
                                        tile_position=(0, 32 * i),
                                    )
                            for j in range(2):
                                i = 2 * pr + j
                                for cp in range(2):
                                    kc = KC[cp]
                                    nc.tensor.matmul(
                                        z_ps[32 * i:32 * (i + 1), 0:N],
                                        lhsT=mones_sb[0:kc, w_abs, cp, :],
                                        rhs=p_sb[0:kc, j,
                                                 cp * N:(cp + 1) * N],
                                        start=(cp == 0), stop=(cp == 1),
                                        tile_position=(0, 32 * i),
                                    )
                        rz = rz_pool.tile([128, N], F32, tag="rz")
                        nc.vector.reciprocal(out=rz[:], in_=z_ps[:, 0:N])
                        nc.vector.tensor_mul(
                            out=oT[:, hg, :], in0=o_ps[:, 0:N], in1=rz[:])
                    # proj for this window
                    NQ = N // 2
                    for qc in range(2):
                        y_ps = ps_a.tile([128, 512], F32, tag="ps_a")
                        for hg in range(4):
                            nc.tensor.matmul(
                                y_ps[0:NQ, :],
                                lhsT=oT[:, hg, qc * NQ:(qc + 1) * NQ],
                                rhs=wp_sb[:, hg, :],
                                start=(hg == 0), stop=(hg == 3),
                            )
                        y_sb = y_pool.tile([NQ, DIM], F32, tag="y")
                        nc.vector.tensor_add(
                            out=y_sb[:], in0=y_ps[0:NQ, :],
                            in1=bp_bc[0:NQ, :])
                        nc.sync.dma_start(
                            out=out_d[w_abs, qc * NQ:(qc + 1) * NQ, :],
                            in_=y_sb[:],
                        )
                    continue
                for hg in range(4):
                    s_ps = ps_s.tile([128, 4, 512], F32, tag="s")
                    # rpe bias copy via ident matmul (ident stays loaded
                    # across the 4 heads of each chunk)
                    # NOTE: start=True (first_mm) clears the WHOLE PSUM bank,
                    # so only the first MM touching each bank may set it;
                    # later MMs rely on has_written=0 -> direct write.
                    if variant != "nobias":
                        for cp in range(2):
                            kc = KC[cp]
                            for i in range(4):
                                nc.tensor.matmul(
                                    s_ps[0:kc, i, cp * N:(cp + 1) * N],
                                    lhsT=ident_sb[0:kc, 0:kc],
                                    rhs=biasT_sb[0:kc, cp, :, 4 * hg + i],
                                    start=(cp == 0), stop=False,
                                    skip_group_check=True,
                                )
                    # S^T += K^T q-stream, 4 heads row-packed
                    for cp in range(2):
                        kc = KC[cp]
                        for i in range(4):
                            nc.tensor.matmul(
                                s_ps[0:kc, i, cp * N:(cp + 1) * N],
                                lhsT=qk_sb[32 * i:32 * (i + 1), 4 + hg,
                                           w0 + cp * 128:w0 + cp * 128 + kc],
                                rhs=qk_sb[32 * i:32 * (i + 1), hg,
                                          w0:w0 + N],
                                start=(variant == "nobias" and cp == 0),
                                stop=(cp == 1),
                                tile_position=(32 * i, 0),
                                skip_group_check=True,
                            )
                    # P = exp(S - 4), all 4 banks in one ACT call
                    p_sb = p_pool.tile([128, 4, 2 * N], F16, tag="p")
                    if variant == "expsplit":
                        for cp in range(2):
                            kc = KC[cp]
                            nc.scalar.activation(
                                out=p_sb[0:kc, :, cp * N:(cp + 1) * N],
                                in_=s_ps[0:kc, :, cp * N:(cp + 1) * N],
                                func=mybir.ActivationFunctionType.Exp,
                                bias=shift_sb[0:kc, 0:1],
                                scale=1.0,
                            )
                    else:
                        nc.scalar.activation(
                            out=p_sb[:],
                            in_=s_ps[:, :, 0:2 * N],
                            func=mybir.ActivationFunctionType.Exp,
                            bias=shift_sb[:, 0:1],
                            scale=1.0,
                        )
                    # O^T = V'^T P col-packed; Z via mask-columns
                    o_ps = ps_a.tile([128, 512], F32, tag="ps_a")
                    z_ps = ps_a.tile([128, 512], F32, tag="ps_a")
                    for i in range(4):
                        h = 4 * hg + i
                        for cp in range(2):
                            kc = KC[cp]
                            nc.tensor.matmul(
                                o_ps[32 * i:32 * (i + 1), 0:N],
                                lhsT=v_sb[0:kc, wi, cp, 32 * h:32 * (h + 1)],
                                rhs=p_sb[0:kc, i, cp * N:(cp + 1) * N],
                                start=(cp == 0), stop=(cp == 1),
                                tile_position=(0, 32 * i),
                            )
                    for i in range(4):
                        for cp in range(2):
                            kc = KC[cp]
                            nc.tensor.matmul(
                                z_ps[32 * i:32 * (i + 1), 0:N],
                                lhsT=mones_sb[0:kc, w_abs, cp, :],
                                rhs=p_sb[0:kc, i, cp * N:(cp + 1) * N],
                                start=(cp == 0), stop=(cp == 1),
                                tile_position=(0, 32 * i),
                            )
                    rz = rz_pool.tile([128, N], F32, tag="rz")
                    nc.vector.reciprocal(out=rz[:], in_=z_ps[:, 0:N])
                    nc.vector.tensor_mul(
                        out=oT[:, hg, :], in0=o_ps[:, 0:N], in1=rz[:])

                # ---------------- proj ----------------
                NQ = N // 2
                for qc in range(2):
                    y_ps = ps_a.tile([128, 512], F32, tag="ps_a")
                    for hg in range(4):
                        nc.tensor.matmul(
                            y_ps[0:NQ, :],
                            lhsT=oT[:, hg, qc * NQ:(qc + 1) * NQ],
                            rhs=wp_sb[:, hg, :],
                            start=(hg == 0), stop=(hg == 3),
                        )
                    y_sb = y_pool.tile([NQ, DIM], F32, tag="y")
                    nc.vector.tensor_add(
                        out=y_sb[:], in0=y_ps[0:NQ, :], in1=bp_bc[0:NQ, :])
                    nc.sync.dma_start(
                        out=out_d[w_abs, qc * NQ:(qc + 1) * NQ, :],
                        in_=y_sb[:],
                    )
    nc.compile()
    return nc


def _host_prep(x, rpe_index, mask, qkv_w, qkv_b, proj_w, proj_b, rpe_table,
               n_w=W, n_cores=NCORES):
    """Shard + layout/dtype prep (numpy only). Returns per-core input maps."""
    x = np.asarray(x, dtype=np.float32)
    rpe_index = np.asarray(rpe_index).astype(np.int64)
    mask = np.asarray(mask).astype(np.int32)
    qkv_w = np.asarray(qkv_w, dtype=np.float32)
    qkv_b = np.asarray(qkv_b, dtype=np.float32)
    proj_w = np.asarray(proj_w, dtype=np.float32)
    proj_b = np.asarray(proj_b, dtype=np.float32)
    rpe_table = np.asarray(rpe_table, dtype=np.float32)

    scale = HD ** -0.5
    wq = qkv_w[0:DIM] * scale
    wk = qkv_w[DIM:2 * DIM]
    wv = qkv_w[2 * DIM:3 * DIM]
    wqk_t = np.concatenate([wq, wk], axis=0).T.astype(np.float16)
    wv_t = wv.T.astype(np.float16)
    wp_t = proj_w.T.astype(np.float16)
    wqk_t = np.ascontiguousarray(
        wqk_t.reshape(4, 128, 2 * DIM).transpose(1, 0, 2).reshape(128, -1))
    wv_t = np.ascontiguousarray(
        wv_t.reshape(4, 128, DIM).transpose(1, 0, 2).reshape(128, -1))
    wp_t = np.ascontiguousarray(
        wp_t.reshape(4, 128, DIM).transpose(1, 0, 2).reshape(128, -1))

    bqk = np.concatenate([qkv_b[0:DIM] * scale, qkv_b[DIM:2 * DIM]])
    bqk_pp = np.ascontiguousarray(
        bqk.reshape(8, 128).T.astype(np.float32))
    bv = qkv_b[2 * DIM:3 * DIM].astype(np.float32)

    # host-side RPE gather into S^T layout: biasT[p, c', q, h], k = 128c'+p
    idx2 = rpe_index.reshape(N, N)                        # [q, k]
    tab16 = rpe_table.astype(np.float16)                  # [729, 16]
    biasT = np.zeros((128, 2, N, H), dtype=np.float16)
    for cp in range(2):
        kc = KC[cp]
        k = 128 * cp + np.arange(kc)
        g = tab16[idx2[:, k], :]                          # [q, kc, H]
        biasT[0:kc, cp] = g.transpose(1, 0, 2)            # [kc, q, H]
    biasT = np.ascontiguousarray(biasT.reshape(128, 2 * N * H))

    ident = np.eye(128, dtype=np.float16)

    in_maps = []
    for core in range(n_cores):
        xs = x[core * n_w:(core + 1) * n_w]
        ms = mask[core * n_w:(core + 1) * n_w].astype(np.float32)  # [n_w, N]
        mones = np.zeros((128, n_w, 2, HD), dtype=np.float16)
        maskv = np.zeros((128, n_w, 2), dtype=np.float32)
        for cp in range(2):
            kc = KC[cp]
            k = 128 * cp + np.arange(kc)
            mones[0:kc, :, cp, :] = ms.T[k][:, :, None].astype(np.float16)
            maskv[0:kc, :, cp] = ms.T[k]
        in_maps.append({
            "x": np.ascontiguousarray(xs),
            "wqk": wqk_t, "wv": wv_t, "wp": wp_t,
            "bqk": bqk_pp, "bv": bv, "bp": proj_b.astype(np.float32),
            "biasT": biasT,
            "mones": np.ascontiguousarray(mones.reshape(128, n_w * 2 * HD)),
            "maskv": np.ascontiguousarray(maskv.reshape(128, n_w * 2)),
            "ident": ident,
        })
    return in_maps


_NC_CACHE = {}


def kernel(x, rpe_index, mask, qkv_w, qkv_b, proj_w, proj_b, rpe_table,
           _trace=False):
    from concourse.bass_utils import run_bass_kernel_spmd
    in_maps = _host_prep(x, rpe_index, mask, qkv_w, qkv_b, proj_w, proj_b,
                         rpe_table)
    if "nc" not in _NC_CACHE:
        _NC_CACHE["nc"] = _build_nc()
    nc = _NC_CACHE["nc"]
    try:
        res = run_bass_kernel_spmd(nc, in_maps, core_ids=list(range(NCORES)),
                                   trace=_trace)
    except ModuleNotFoundError:
        res = run_bass_kernel_spmd(nc, in_maps, core_ids=list(range(NCORES)),
                                   trace=False)
    kernel.last_results = res
    out = np.concatenate([r["out"] for r in res.results], axis=0)
    return out.reshape(B, N, DIM).astype(np.float32)


# revision 6
# speedup vs baseline: 1.0001x; 1.0001x over previous
"""Trainium2 Bass kernel for nn_Attention_4045859193206 (Swin-style window
attention with relative position bias + key masking).

Contract: kernel(**inputs) takes FULL inputs (B=128 windows), shards the
batch across 8 NeuronCores (16 windows each), runs one SPMD Bass kernel,
returns the FULL (128, 196, 512) float32 output. Self-contained.

Design (variant "pair2"; TimelineSim 318us/core vs 492us for the staged v1):
  - RPE bias gathered on HOST (numpy) and shipped as a [128, 2, 196, 16]
    fp16 table in S^T layout; kills the on-device gpsimd dma_gather
    (~71us DMA + gather) and the idx stream.
  - Key mask applied by zeroing V rows and using mask-columns instead of
    ones for the Z (denominator) matmul -- exactly equivalent to -inf
    masking, and makes exp() window-independent (constant -4.0 bias only).
  - S^T computed per HEAD PAIR into 2-bank PSUM tiles, double-buffered
    (ps_s bufs=2), so each pair's ScalarE exp [128, 2, 392] overlaps the
    next pair's bias+QK matmuls. o/z PSUM pool split from the mm/y pool so
    attention tiles don't contend with the next group's QKV tiles.
  - k-chunks are (128, 68) instead of (98, 98): QK/V c0 chunks get
    M=128 stationary tiles (fast-weight-load eligible).
  - bias injected by full-array ident matmuls (start=True opens each bank
    ONCE -- first_mm clears the whole bank; QK then accumulates with
    start=False). The full-array ident also serializes bias before the
    row-packed QK on HW; packed-bias alternatives race (overlapping
    outputs from concurrent tiles) and produce NaN.
  - Q^T/K^T built per group-half in separate tiles so windows 0-1 of each
    4-window group unblock after half the QKV matmuls; QKV bias adds on
    VectorE (nc.any put them on the exp-critical ScalarE).
  - fp16 everywhere: fp8 measured infeasible (0.02..0.07 rel err vs the
    2e-2 gate on every stage combination).
"""

import contextlib
import numpy as np

import concourse.bass as bass
import concourse.mybir as mybir
import concourse.tile as tile
from concourse.bacc import Bacc

# ---------------------------------------------------------------- constants
B, N, DIM, H = 128, 196, 512, 16
HD = DIM // H                     # 32
RPE = 729
NCORES = 8
W = B // NCORES                   # 16 windows per core
GW = 4                            # windows per qkv group (4*196=784 tokens)
KC = (128, 68)                    # k chunks per window
F16 = mybir.dt.float16
F32 = mybir.dt.float32
EXP_SHIFT = -4.0                  # exp(s-4): fp16 headroom; cancels in softmax


def _build_nc(n_w=W, variant="pair2", bufs=None):
    _b = dict(xt=2, qk=4, v=2, p=4, o=3, y=3, rz=4)
    _b.update(bufs or {})
    bufs = _b
    assert n_w % GW == 0
    ngrp = n_w // GW
    nc = Bacc("TRN2", target_bir_lowering=False)

    x_d = nc.dram_tensor("x", (n_w, N, DIM), F32, kind="ExternalInput")
    wqk_d = nc.dram_tensor("wqk", (128, 4 * 2 * DIM), F16, kind="ExternalInput")
    wv_d = nc.dram_tensor("wv", (128, 4 * DIM), F16, kind="ExternalInput")
    wp_d = nc.dram_tensor("wp", (128, 4 * DIM), F16, kind="ExternalInput")
    bqk_d = nc.dram_tensor("bqk", (128, 8), F32, kind="ExternalInput")
    bv_d = nc.dram_tensor("bv", (DIM,), F32, kind="ExternalInput")
    bp_d = nc.dram_tensor("bp", (DIM,), F32, kind="ExternalInput")
    biasT_d = nc.dram_tensor("biasT", (128, 2 * N * H), F16,
                             kind="ExternalInput")
    mones_d = nc.dram_tensor("mones", (128, n_w * 2 * HD), F16,
                             kind="ExternalInput")
    maskv_d = nc.dram_tensor("maskv", (128, n_w * 2), F32,
                             kind="ExternalInput")
    ident_d = nc.dram_tensor("ident", (128, 128), F16, kind="ExternalInput")
    out_d = nc.dram_tensor("out", (n_w, N, DIM), F32, kind="ExternalOutput")

    x16_d = nc.dram_tensor("x16", (n_w * N, DIM), F16)

    with tile.TileContext(nc) as tc, contextlib.ExitStack() as ctx:
        const = ctx.enter_context(tc.tile_pool(name="const", bufs=1))
        xt_pool = ctx.enter_context(tc.tile_pool(name="xt", bufs=bufs["xt"]))
        qk_pool = ctx.enter_context(tc.tile_pool(name="qk", bufs=bufs["qk"]))
        v_pool = ctx.enter_context(tc.tile_pool(name="v", bufs=bufs["v"]))
        p_pool = ctx.enter_context(tc.tile_pool(name="p", bufs=bufs["p"]))
        o_pool = ctx.enter_context(tc.tile_pool(name="o", bufs=bufs["o"]))
        y_pool = ctx.enter_context(tc.tile_pool(name="y", bufs=bufs["y"]))
        rz_pool = ctx.enter_context(tc.tile_pool(name="rz", bufs=bufs["rz"]))
        ps_s = ctx.enter_context(tc.tile_pool(
            name="ps_s", bufs=(2 if variant in ("pair", "pair2", "pair3") else 1), space="PSUM"))
        ps_a = ctx.enter_context(tc.tile_pool(
            name="ps_a", bufs=(2 if variant in ("pair2", "pair3") else 4), space="PSUM"))
        ps_oz = ctx.enter_context(tc.tile_pool(
            name="ps_oz", bufs=2, space="PSUM")) if variant in ("pair2", "pair3") else ps_a

        # ---------------- constants ----------------
        wqk_sb = const.tile([128, 4, 2 * DIM], F16)   # [c128, ci, o] (q|k)
        wv_sb = const.tile([128, 4, DIM], F16)
        wp_sb = const.tile([128, 4, DIM], F16)
        nc.sync.dma_start(out=wqk_sb, in_=wqk_d[:].rearrange("b (a c) -> b a c", a=4))
        nc.sync.dma_start(out=wv_sb, in_=wv_d[:].rearrange("b (a c) -> b a c", a=4))
        nc.sync.dma_start(out=wp_sb, in_=wp_d[:].rearrange("b (a c) -> b a c", a=4))
        ident_sb = const.tile([128, 128], F16)
        nc.sync.dma_start(out=ident_sb, in_=ident_d[:])
        bqk_sb = const.tile([128, 8], F32)            # per-partition qk bias
        nc.sync.dma_start(out=bqk_sb, in_=bqk_d[:])
        bv_bc = const.tile([128, DIM], F32)           # broadcast rows
        nc.sync.dma_start(
            out=bv_bc, in_=bass.AP(tensor=bv_d[:].tensor, offset=0,
                                   ap=[[0, 128], [1, DIM]]))
        bp_bc = const.tile([128, DIM], F32)
        nc.sync.dma_start(
            out=bp_bc, in_=bass.AP(tensor=bp_d[:].tensor, offset=0,
                                   ap=[[0, 128], [1, DIM]]))
        # biasT_sb[p, c', q, h] = rpe bias for k = 128c'+p
        biasT_sb = const.tile([128, 2, N, H], F16)
        nc.sync.dma_start(
            out=biasT_sb,
            in_=biasT_d[:].rearrange("p (c q h) -> p c q h", c=2, q=N))
        mones_sb = const.tile([128, n_w, 2, HD], F16)
        nc.sync.dma_start(
            out=mones_sb,
            in_=mones_d[:].rearrange("p (w c d) -> p w c d", w=n_w, c=2))
        maskv_sb = const.tile([128, n_w, 2], F32)
        nc.sync.dma_start(
            out=maskv_sb,
            in_=maskv_d[:].rearrange("p (w c) -> p w c", w=n_w))
        shift_sb = const.tile([128, 1], F32)
        nc.vector.memset(shift_sb, EXP_SHIFT)

        # ---------------- main loop over 4-window groups ----------------
        for g in range(ngrp):
            tok0 = g * GW * N
            nc.gpsimd.dma_start(
                out=x16_d[tok0:tok0 + GW * N, :],
                in_=x_d[:].rearrange("w n c -> (w n) c")[tok0:tok0 + GW * N, :],
            )
            xt = xt_pool.tile([128, 4, GW * N], F16, tag="xt")
            for ci in range(4):
                nc.sync.dma_start_transpose(
                    out=xt[:, ci, :],
                    in_=x16_d[tok0:tok0 + GW * N, ci * 128:(ci + 1) * 128],
                )

            # Q^T / K^T  [o-chunk 128, tok] fp16 (q pre-scaled via weights);
            # one tile per group-half so windows 0-1 unblock after half the
            # QKV matmuls
            qkh = [qk_pool.tile([128, 8, 2 * N], F16, tag=f"qk{h}",
                                name=f"qkh{h}") for h in range(2)]
            for half in range(2):
                for oc in range(8):
                    mm_ps = ps_a.tile([128, 512], F32, tag="ps_a")
                    for ci in range(4):
                        nc.tensor.matmul(
                            mm_ps[:, 0:392],
                            lhsT=wqk_sb[:, ci, oc * 128:(oc + 1) * 128],
                            rhs=xt[:, ci, half * 392:(half + 1) * 392],
                            start=(ci == 0), stop=(ci == 3),
                        )
                    nc.vector.tensor_scalar_add(
                        out=qkh[half][:, oc, :],
                        in0=mm_ps[:, 0:392],
                        scalar1=bqk_sb[:, oc:oc + 1],
                    )

            # V natural [k-chunk, 512] fp16, bias-added then mask-zeroed
            v_sb = v_pool.tile([128, GW, 2, DIM], F16, tag="v")
            for wi in range(GW):
                w_abs = g * GW + wi
                for cp in range(2):
                    kc = KC[cp]
                    vv_ps = ps_a.tile([128, 512], F32, tag="ps_a")
                    for ci in range(4):
                        nc.tensor.matmul(
                            vv_ps[0:kc, :],
                            lhsT=xt[:, ci, wi * N + cp * 128:
                                    wi * N + cp * 128 + kc],
                            rhs=wv_sb[:, ci, :],
                            start=(ci == 0), stop=(ci == 3),
                        )
                    nc.vector.tensor_add(
                        out=v_sb[0:kc, wi, cp, :],
                        in0=vv_ps[0:kc, :],
                        in1=bv_bc[0:kc, :],
                    )
                    nc.vector.tensor_scalar_mul(
                        out=v_sb[0:kc, wi, cp, :],
                        in0=v_sb[0:kc, wi, cp, :],
                        scalar1=maskv_sb[0:kc, w_abs, cp:cp + 1],
                    )

            # ---------------- attention per window ----------------
            for wi in range(GW):
                w_abs = g * GW + wi
                oT = o_pool.tile([128, 4, N], F16, tag="oT")
                qk_sb = qkh[wi // 2]
                w0 = (wi % 2) * N
                if variant in ("pair", "pair2", "pair3"):
                    # head pairs: 2-bank S tiles, double-buffered so the
                    # next pair's PE matmuls overlap this pair's exp()
                    for hg in range(4):
                        o_ps = ps_oz.tile([128, 512], F32, tag="ps_oz")
                        z_ps = ps_oz.tile([128, 512], F32, tag="ps_oz")
                        for pr in range(2):
                            s_ps = ps_s.tile([128, 2, 512], F32, tag="s")
                            for cp in range(2):
                                kc = KC[cp]
                                for j in range(2):
                                    i = 2 * pr + j
                                    nc.tensor.matmul(
                                        s_ps[0:kc, j, cp * N:(cp + 1) * N],
                                        lhsT=ident_sb[0:kc, 0:kc],
                                        rhs=biasT_sb[0:kc, cp, :, 4 * hg + i],
                                        start=(cp == 0), stop=False,
                                        skip_group_check=True,
                                    )
                            for cp in range(2):
                                kc = KC[cp]
                                for j in range(2):
                                    i = 2 * pr + j
                                    nc.tensor.matmul(
                                        s_ps[0:kc, j, cp * N:(cp + 1) * N],
                                        lhsT=qk_sb[32 * i:32 * (i + 1),
                                                   4 + hg,
                                                   w0 + cp * 128:
                                                   w0 + cp * 128 + kc],
                                        rhs=qk_sb[32 * i:32 * (i + 1), hg,
                                                  w0:w0 + N],
                                        start=False, stop=(cp == 1),
                                        tile_position=(32 * i, 0),
                                        skip_group_check=True,
                                    )
                            p_sb = p_pool.tile([128, 2, 2 * N], F16, tag="p")
                            if variant == "pair3":
                                for cp in range(2):
                                    kc = KC[cp]
                                    nc.scalar.activation(
                                        out=p_sb[0:kc, :,
                                                 cp * N:(cp + 1) * N],
                                        in_=s_ps[0:kc, :,
                                                 cp * N:(cp + 1) * N],
                                        func=mybir.ActivationFunctionType.Exp,
                                        bias=shift_sb[0:kc, 0:1],
                                        scale=1.0,
                                    )
                            else:
                                nc.scalar.activation(
                                    out=p_sb[:],
                                    in_=s_ps[:, :, 0:2 * N],
                                    func=mybir.ActivationFunctionType.Exp,
                                    bias=shift_sb[:, 0:1],
                                    scale=1.0,
                                )
                            for j in range(2):
                                i = 2 * pr + j
                                h = 4 * hg + i
                                for cp in range(2):
                                    kc = KC[cp]
                                    nc.tensor.matmul(
                                        o_ps[32 * i:32 * (i + 1), 0:N],
                                        lhsT=v_sb[0:kc, wi, cp,
                                                  32 * h:32 * (h + 1)],
                                        rhs=p_sb[0:kc, j,
                                                 cp * N:(cp + 1) * N],
                                        start=(cp == 0), stop=(cp == 1),
                                        tile_position=(0, 32 * i),
                                    )
                            for j in range(2):
                                i = 2 * pr + j
                                for cp in range(2):
                                    kc = KC[cp]
                                    nc.tensor.matmul(
                                        z_ps[32 * i:32 * (i + 1), 0:N],
                                        lhsT=mones_sb[0:kc, w_abs, cp, :],
                                        rhs=p_sb[0:kc, j,
                                                 cp * N:(cp + 1) * N],
                                        start=(cp == 0), stop=(cp == 1),
                                        tile_position=(0, 32 * i),
                                    )
                        rz = rz_pool.tile([128, N], F32, tag="rz")
                        nc.vector.reciprocal(out=rz[:], in_=z_ps[:, 0:N])
                        nc.vector.tensor_mul(
                            out=oT[:, hg, :], in0=o_ps[:, 0:N], in1=rz[:])
                    # proj for this window
                    NQ = N // 2
                    for qc in range(2):
                        y_ps = ps_a.tile([128, 512], F32, tag="ps_a")
                        for hg in range(4):
                            nc.tensor.matmul(
                                y_ps[0:NQ, :],
                                lhsT=oT[:, hg, qc * NQ:(qc + 1) * NQ],
                                rhs=wp_sb[:, hg, :],
                                start=(hg == 0), stop=(hg == 3),
                            )
                        y_sb = y_pool.tile([NQ, DIM], F32, tag="y")
                        nc.vector.tensor_add(
                            out=y_sb[:], in0=y_ps[0:NQ, :],
                            in1=bp_bc[0:NQ, :])
                        nc.sync.dma_start(
                            out=out_d[w_abs, qc * NQ:(qc + 1) * NQ, :],
                            in_=y_sb[:],
                        )
                    continue
                for hg in range(4):
                    s_ps = ps_s.tile([128, 4, 512], F32, tag="s")
                    # rpe bias copy via ident matmul (ident stays loaded
                    # across the 4 heads of each chunk)
                    # NOTE: start=True (first_mm) clears the WHOLE PSUM bank,
                    # so only the first MM touching each bank may set it;
                    # later MMs rely on has_written=0 -> direct write.
                    if variant != "nobias":
                        for cp in range(2):
                            kc = KC[cp]
                            for i in range(4):
                                nc.tensor.matmul(
                                    s_ps[0:kc, i, cp * N:(cp + 1) * N],
                                    lhsT=ident_sb[0:kc, 0:kc],
                                    rhs=biasT_sb[0:kc, cp, :, 4 * hg + i],
                                    start=(cp == 0), stop=False,
                                    skip_group_check=True,
                                )
                    # S^T += K^T q-stream, 4 heads row-packed
                    for cp in range(2):
                        kc = KC[cp]
                        for i in range(4):
                            nc.tensor.matmul(
                                s_ps[0:kc, i, cp * N:(cp + 1) * N],
                                lhsT=qk_sb[32 * i:32 * (i + 1), 4 + hg,
                                           w0 + cp * 128:w0 + cp * 128 + kc],
                                rhs=qk_sb[32 * i:32 * (i + 1), hg,
                                          w0:w0 + N],
                                start=(variant == "nobias" and cp == 0),
                                stop=(cp == 1),
                                tile_position=(32 * i, 0),
                                skip_group_check=True,
                            )
                    # P = exp(S - 4), all 4 banks in one ACT call
                    p_sb = p_pool.tile([128, 4, 2 * N], F16, tag="p")
                    if variant == "expsplit":
                        for cp in range(2):
                            kc = KC[cp]
                            nc.scalar.activation(
                                out=p_sb[0:kc, :, cp * N:(cp + 1) * N],
                                in_=s_ps[0:kc, :, cp * N:(cp + 1) * N],
                                func=mybir.ActivationFunctionType.Exp,
                                bias=shift_sb[0:kc, 0:1],
                                scale=1.0,
                            )
                    else:
                        nc.scalar.activation(
                            out=p_sb[:],
                            in_=s_ps[:, :, 0:2 * N],
                            func=mybir.ActivationFunctionType.Exp,
                            bias=shift_sb[:, 0:1],
                            scale=1.0,
                        )
                    # O^T = V'^T P col-packed; Z via mask-columns
                    o_ps = ps_a.tile([128, 512], F32, tag="ps_a")
                    z_ps = ps_a.tile([128, 512], F32, tag="ps_a")
                    for i in range(4):
                        h = 4 * hg + i
                        for cp in range(2):
                            kc = KC[cp]
                            nc.tensor.matmul(
                                o_ps[32 * i:32 * (i + 1), 0:N],
                                lhsT=v_sb[0:kc, wi, cp, 32 * h:32 * (h + 1)],
                                rhs=p_sb[0:kc, i, cp * N:(cp + 1) * N],
                                start=(cp == 0), stop=(cp == 1),
                                tile_position=(0, 32 * i),
                            )
                    for i in range(4):
                        for cp in range(2):
                            kc = KC[cp]
                            nc.tensor.matmul(
                                z_ps[32 * i:32 * (i + 1), 0:N],
                                lhsT=mones_sb[0:kc, w_abs, cp, :],
                                rhs=p_sb[0:kc, i, cp * N:(cp + 1) * N],
                                start=(cp == 0), stop=(cp == 1),
                                tile_position=(0, 32 * i),
                            )
                    rz = rz_pool.tile([128, N], F32, tag="rz")
                    nc.vector.reciprocal(out=rz[:], in_=z_ps[:, 0:N])
                    nc.vector.tensor_mul(
                        out=oT[:, hg, :], in0=o_ps[:, 0:N], in1=rz[:])

                # ---------------- proj ----------------
                NQ = N // 2
                for qc in range(2):
                    y_ps = ps_a.tile([128, 512], F32, tag="ps_a")
                    for hg in range(4):
                        nc.tensor.matmul(
                            y_ps[0:NQ, :],
                            lhsT=oT[:, hg, qc * NQ:(qc + 1) * NQ],
                            rhs=wp_sb[:, hg, :],
                            start=(hg == 0), stop=(hg == 3),
                        )
                    y_sb = y_pool.tile([NQ, DIM], F32, tag="y")
                    nc.vector.tensor_add(
                        out=y_sb[:], in0=y_ps[0:NQ, :], in1=bp_bc[0:NQ, :])
                    nc.sync.dma_start(
                        out=out_d[w_abs, qc * NQ:(qc + 1) * NQ, :],
                        in_=y_sb[:],
                    )
    nc.compile()
    return nc


def _host_prep(x, rpe_index, mask, qkv_w, qkv_b, proj_w, proj_b, rpe_table,
               n_w=W, n_cores=NCORES):
    """Shard + layout/dtype prep (numpy only). Returns per-core input maps."""
    x = np.asarray(x, dtype=np.float32)
    rpe_index = np.asarray(rpe_index).astype(np.int64)
    mask = np.asarray(mask).astype(np.int32)
    qkv_w = np.asarray(qkv_w, dtype=np.float32)
    qkv_b = np.asarray(qkv_b, dtype=np.float32)
    proj_w = np.asarray(proj_w, dtype=np.float32)
    proj_b = np.asarray(proj_b, dtype=np.float32)
    rpe_table = np.asarray(rpe_table, dtype=np.float32)

    scale = HD ** -0.5
    wq = qkv_w[0:DIM] * scale
    wk = qkv_w[DIM:2 * DIM]
    wv = qkv_w[2 * DIM:3 * DIM]
    wqk_t = np.concatenate([wq, wk], axis=0).T.astype(np.float16)
    wv_t = wv.T.astype(np.float16)
    wp_t = proj_w.T.astype(np.float16)
    wqk_t = np.ascontiguousarray(
        wqk_t.reshape(4, 128, 2 * DIM).transpose(1, 0, 2).reshape(128, -1))
    wv_t = np.ascontiguousarray(
        wv_t.reshape(4, 128, DIM).transpose(1, 0, 2).reshape(128, -1))
    wp_t = np.ascontiguousarray(
        wp_t.reshape(4, 128, DIM).transpose(1, 0, 2).reshape(128, -1))

    bqk = np.concatenate([qkv_b[0:DIM] * scale, qkv_b[DIM:2 * DIM]])
    bqk_pp = np.ascontiguousarray(
        bqk.reshape(8, 128).T.astype(np.float32))
    bv = qkv_b[2 * DIM:3 * DIM].astype(np.float32)

    # host-side RPE gather into S^T layout: biasT[p, c', q, h], k = 128c'+p
    idx2 = rpe_index.reshape(N, N)                        # [q, k]
    tab16 = rpe_table.astype(np.float16)                  # [729, 16]
    biasT = np.zeros((128, 2, N, H), dtype=np.float16)
    for cp in range(2):
        kc = KC[cp]
        k = 128 * cp + np.arange(kc)
        g = tab16[idx2[:, k], :]                          # [q, kc, H]
        biasT[0:kc, cp] = g.transpose(1, 0, 2)            # [kc, q, H]
    biasT = np.ascontiguousarray(biasT.reshape(128, 2 * N * H))

    ident = np.eye(128, dtype=np.float16)

    in_maps = []
    for core in range(n_cores):
        xs = x[core * n_w:(core + 1) * n_w]
        ms = mask[core * n_w:(core + 1) * n_w].astype(np.float32)  # [n_w, N]
        mones = np.zeros((128, n_w, 2, HD), dtype=np.float16)
        maskv = np.zeros((128, n_w, 2), dtype=np.float32)
        for cp in range(2):
            kc = KC[cp]
            k = 128 * cp + np.arange(kc)
            mones[0:kc, :, cp, :] = ms.T[k][:, :, None].astype(np.float16)
            maskv[0:kc, :, cp] = ms.T[k]
        in_maps.append({
            "x": np.ascontiguousarray(xs),
            "wqk": wqk_t, "wv": wv_t, "wp": wp_t,
            "bqk": bqk_pp, "bv": bv, "bp": proj_b.astype(np.float32),
            "biasT": biasT,
            "mones": np.ascontiguousarray(mones.reshape(128, n_w * 2 * HD)),
            "maskv": np.ascontiguousarray(maskv.reshape(128, n_w * 2)),
            "ident": ident,
        })
    return in_maps


_NC_CACHE = {}


def kernel(x, rpe_index, mask, qkv_w, qkv_b, proj_w, proj_b, rpe_table,
           _trace=False):
    from concourse.bass_utils import run_bass_kernel_spmd
    in_maps = _host_prep(x, rpe_index, mask, qkv_w, qkv_b, proj_w, proj_b,
                         rpe_table)
    if "nc" not in _NC_CACHE:
        _NC_CACHE["nc"] = _build_nc()
    nc = _NC_CACHE["nc"]
    try:
        res = run_bass_kernel_spmd(nc, in_maps, core_ids=list(range(NCORES)),
                                   trace=_trace)
    except ModuleNotFoundError:
        res = run_bass_kernel_spmd(nc, in_maps, core_ids=list(range(NCORES)),
                                   trace=False)
    kernel.last_results = res
    out = np.concatenate([r["out"] for r in res.results], axis=0)
    return out.reshape(B, N, DIM).astype(np.float32)
